# revision 15
# baseline (speedup 1.0000x reference)
"""GroupedQueryAttention Trainium2 Bass kernel (8 NeuronCores, SPMD).

Reference quirk exploited: K/V are tiled R=4x along the group axis and
attention runs over the full concatenated 2048-dim. Mathematically this
collapses:
  scores = Q . tile(K)  ==  (sum of Q's four 512-chunks) . K      (512-dim)
  Z      = attn . tile(V)  -> tiled copies of  attn . V           (512-dim)
  out    = Z @ proj     ==  (attn . V) @ (sum of proj's 4 row-blocks)
So the whole module reduces to a single 512-dim attention:
  Qc = x @ WQc.T + bQc   (WQc = sum of WQ row-blocks)
  K  = x @ WK.T + bK ; V = x @ WV.T + bV
  S  = Qc K^T (causal), softmax, /sqrt(128)
  y  = (softmax(S)/sqrt(128) V) @ projc    (projc = sum of proj row-blocks)
This cuts FLOPs ~3x vs the literal graph.

Sharding: 8 cores = 4 batches x 2 (interleaved 128-row blocks). Core with
pairpos q of batch b owns rows {256g+128q .. 256g+128q+127, g=0..7}. The
causal key extent of block g is tiled at 256 granularity: (g+1)//2 full
512-tiles plus, for even g, a 256 tail. Both cores of a pair see the same
tile shapes -> a single SPMD program; the causal masks (two of them, by
block parity) are per-core input data.

All matmuls run as float32r (full-speed PE mode, ~tf32 accuracy).
Transposes are plain matmuls against a 128x128 identity moving operand
(out = lhsT.T @ I), ~3x faster than PE transpose-mode. Attention
row-blocks are software-pipelined: S(g+1) matmuls hide the zn normalize
of block g, Z^T+O(g) hide exp(g+1), and the output projection's DMA-out
overlaps the remaining blocks.
"""

import numpy as np

import concourse.bacc as bacc
import concourse.mybir as mybir
from concourse.tile import TileContext
from concourse.bass_utils import run_bass_kernel_spmd

B, T, D = 4, 2048, 2048
HD = 512                 # collapsed head dim
NCORES = 8
RB = 8                   # 128-row blocks per core
DCH = D // 128           # 16 contraction chunks
dt = mybir.dt
NEG = -1.0e30


def key_tiles(g):
    """Causal key tiles for row block g: [(col0, width), ...]."""
    t = [(512 * kt, 512) for kt in range((g + 1) // 2)]
    if g % 2 == 0:
        t.append((256 * g, 256))
    return t


def build_kernel():
    nc = bacc.Bacc(None, target_bir_lowering=False)

    # x streams pre-tiled host-side: tile 4*rt+t packs contraction chunks
    # c=4t+i at cols [512i, 512i+512) for key-group rt -> 8KB SBUF rows
    # (DMA queues move ~4x the bytes/packet vs 2KB rows).
    xT_d = nc.dram_tensor("xT", [16, 128, 2048], dt.float32r, kind="ExternalInput")
    xTq_d = nc.dram_tensor("xTq", [8, 128, 2048], dt.float32r, kind="ExternalInput")
    WKT_d = nc.dram_tensor("WKT", [D, HD], dt.float32r, kind="ExternalInput")
    WVT_d = nc.dram_tensor("WVT", [D, HD], dt.float32r, kind="ExternalInput")
    WQT_d = nc.dram_tensor("WQT", [D, HD], dt.float32r, kind="ExternalInput")
    PRJ_d = nc.dram_tensor("PRJ", [HD, D], dt.bfloat16, kind="ExternalInput")
    BIA_d = nc.dram_tensor("BIA", [128, 8], dt.float32, kind="ExternalInput")
    bVr_d = nc.dram_tensor("bVr", [128, HD], dt.float32, kind="ExternalInput")
    ONE_d = nc.dram_tensor("ONE", [1, 128], dt.float32r, kind="ExternalInput")
    MSK_d = nc.dram_tensor("MSK", [2, 128, 512], dt.float32, kind="ExternalInput")
    IDT_d = nc.dram_tensor("IDT", [128, 128], dt.bfloat16, kind="ExternalInput")
    OUT_d = nc.dram_tensor("out", [RB, 128, D], dt.float32, kind="ExternalOutput")

    Act = mybir.ActivationFunctionType
    Ax = mybir.AxisListType

    with TileContext(nc) as tc:
        with tc.tile_pool(name="persist", bufs=1) as pp:
            # ---- persistent tiles ------------------------------------------
            KT_sb = [pp.tile([128, T], dt.float32r, tag=f"KT{h}", name=f"KT{h}")
                     for h in range(4)]
            V_sb = [pp.tile([128, HD], dt.bfloat16, tag=f"V{k}", name=f"V{k}")
                    for k in range(16)]
            # Q^T split by 512-col group so S(0) doesn't wait on the rt=1 write
            QT_sb = [[pp.tile([128, 512], dt.float32r, tag=f"QT{h}{rt}",
                              name=f"QT{h}{rt}") for rt in range(2)]
                     for h in range(4)]
            ident = pp.tile([128, 128], dt.bfloat16, tag="ident")
            ones = pp.tile([1, 128], dt.float32r, tag="ones")
            bvr = pp.tile([128, HD], dt.float32, tag="bvr")
            bia = pp.tile([128, 8], dt.float32, tag="bia")
            msk_sb = [pp.tile([128, 512], dt.float32, tag=f"msk{m}",
                              name=f"msk{m}") for m in range(2)]
            prj = [pp.tile([128, D], dt.bfloat16, tag=f"prj{h}", name=f"prj{h}")
                   for h in range(4)]

            def bias_ap(nm, h):
                col = h if nm == "bK" else 4 + h
                return bia[:, col:col + 1]

            # ---- stage P: projections --------------------------------------
            # x^T streamed by 128-D chunk; weights streamed with the first key
            # group then resident. Per 512-key group: 4 PSUM banks accumulate
            # K^T hd-tiles ([hd, keys], bias per partition) and 4 banks
            # accumulate V key-blocks directly in [keys, hd] layout (xt chunk
            # as lhsT); V bias enters as a rank-1 ones x bV matmul. PSUM
            # drains are split between ScalarE and VectorE.
            with tc.tile_pool(name="wpool", bufs=1) as wp, \
                 tc.tile_pool(name="xstream", bufs=3) as xp, \
                 tc.tile_pool(name="psP", bufs=1, space="PSUM") as psp:
                wk = [None] * DCH
                wv = [None] * DCH
                for rt in range(4):                      # key 512-col groups
                    kps = [psp.tile([128, 512], dt.float32, tag=f"kps{h}",
                                    name=f"kps{h}") for h in range(4)]
                    vps = [psp.tile([128, 512], dt.float32, tag=f"vps{j}",
                                    name=f"vps{j}") for j in range(4)]
                    for t in range(4):
                        xt = xp.tile([128, 2048], dt.float32r, tag="xs", name="xt")
                        if rt == 0 and t == 0:
                            nc.gpsimd.dma_start(out=xt[:, 0:512],
                                                in_=xT_d[0][:, 0:512])
                            nc.gpsimd.dma_start(out=xt[:, 512:2048],
                                                in_=xT_d[0][:, 512:2048])
                        else:
                            nc.gpsimd.dma_start(out=xt[:], in_=xT_d[4 * rt + t])
                        for i in range(4):
                            c = 4 * t + i
                            if rt == 0:   # stream weights in with first pass
                                tk = wp.tile([128, HD], dt.float32r, tag=f"wk{c}",
                                             name=f"wk{c}")
                                nc.sync.dma_start(
                                    out=tk[:], in_=WKT_d[128 * c:128 * c + 128, :])
                                wk[c] = tk
                                tv = wp.tile([128, HD], dt.float32r, tag=f"wv{c}",
                                             name=f"wv{c}")
                                nc.sync.dma_start(
                                    out=tv[:], in_=WVT_d[128 * c:128 * c + 128, :])
                                wv[c] = tv
                                if c == 0:
                                    # tiny constants on the idle scalar queue
                                    nc.scalar.dma_start(out=bia[:], in_=BIA_d[:])
                                    nc.scalar.dma_start(out=ones[:], in_=ONE_d[:])
                                    nc.scalar.dma_start(out=bvr[:], in_=bVr_d[:])
                            st = (c == 0)
                            xsl = xt[:, 512 * i:512 * i + 512]
                            for h in range(4):
                                nc.tensor.matmul(kps[h][:],
                                                 wk[c][:, 128 * h:128 * h + 128],
                                                 xsl, start=st,
                                                 stop=(c == DCH - 1))
                            for j in range(4):
                                nc.tensor.matmul(
                                    vps[j][:],
                                    xt[:, 512 * i + 128 * j:
                                       512 * i + 128 * j + 128],
                                    wv[c][:], start=st,
                                    stop=(c == DCH - 1))
                    if rt == 0:
                        # prefetches ride the DMA slack (first use much later)
                        nc.sync.dma_start(out=ident[:], in_=IDT_d[:])
                        for m in range(2):
                            nc.sync.dma_start(out=msk_sb[m][:], in_=MSK_d[m])
                        for h in range(4):
                            nc.sync.dma_start(out=prj[h][:],
                                              in_=PRJ_d[128 * h:128 * h + 128, :])
                    for h in range(4):
                        if h % 2 == 0:
                            nc.scalar.activation(
                                KT_sb[h][:, 512 * rt:512 * rt + 512], kps[h][:],
                                Act.Identity, bias=bias_ap("bK", h), scale=1.0)
                        else:
                            nc.vector.tensor_scalar_add(
                                KT_sb[h][:, 512 * rt:512 * rt + 512], kps[h][:],
                                bias_ap("bK", h))
                    for j in range(4):   # V bias folded into the drain
                        nc.vector.tensor_add(V_sb[4 * rt + j][:], vps[j][:],
                                             bvr[:])
                # Qc^T for this core's 1024 rows (reuses the wk slots).
                wq = []
                for c in range(DCH):
                    tq = wp.tile([128, HD], dt.float32r, tag=f"wk{c}", name=f"wq{c}")
                    nc.sync.dma_start(out=tq[:], in_=WQT_d[128 * c:128 * c + 128, :])
                    wq.append(tq)
                for rt in range(2):
                    qps = [psp.tile([128, 512], dt.float32, tag=f"kps{h}",
                                    name=f"qps{h}") for h in range(4)]
                    for t in range(4):
                        xt = xp.tile([128, 2048], dt.float32r, tag="xs", name="xt")
                        nc.gpsimd.dma_start(out=xt[:], in_=xTq_d[4 * rt + t])
                        for i in range(4):
                            c = 4 * t + i
                            for h in range(4):
                                nc.tensor.matmul(qps[h][:],
                                                 wq[c][:, 128 * h:128 * h + 128],
                                                 xt[:, 512 * i:512 * i + 512],
                                                 start=(c == 0),
                                                 stop=(c == DCH - 1))
                    for h in range(4):
                        if h % 2 == 0:
                            nc.scalar.activation(
                                QT_sb[h][rt][:], qps[h][:],
                                Act.Identity, bias=bias_ap("bQ", h), scale=1.0)
                        else:
                            nc.vector.tensor_scalar_add(
                                QT_sb[h][rt][:], qps[h][:], bias_ap("bQ", h))

            # ---- stages A+O: attention fused with output projection --------
            with tc.tile_pool(name="attn", bufs=1) as ap, \
                 tc.tile_pool(name="psA", bufs=1, space="PSUM") as psa:
                state = {}

                def qt_ap(h, g):
                    return QT_sb[h][g // 4][:, 128 * (g % 4):128 * (g % 4) + 128]

                def stage_s(g):
                    """S matmuls + mask + per-tile max for row block g."""
                    tl = key_tiles(g)
                    mpart = ap.tile([128, 4], dt.float32, tag="mpart", bufs=2,
                                    name="mpart")
                    s_tiles = []
                    for i, (col0, w) in enumerate(tl):
                        sps = psa.tile([128, w], dt.float32, tag="sps", bufs=2,
                                       name="sps")
                        for h in range(4):
                            nc.tensor.matmul(
                                sps[:], qt_ap(h, g),
                                KT_sb[h][:, col0:col0 + w],
                                start=(h == 0), stop=(h == 3))
                        # exp reads these as fp32r (same bits) so the E^T
                        # transpose can be a plain full-speed matmul
                        ssb = ap.tile([128, w], dt.float32, tag="ssb", bufs=9,
                                      name="ssb")
                        if i == len(tl) - 1:
                            nc.vector.tensor_add(ssb[:], sps[:],
                                                 msk_sb[g % 2][:, 0:w])
                        elif i % 2 == 0:
                            nc.scalar.copy(ssb[:], sps[:])
                        else:
                            nc.vector.tensor_copy(ssb[:], sps[:])
                        nc.vector.reduce_max(mpart[:, i:i + 1], ssb[:],
                                             axis=Ax.X)
                        s_tiles.append(ssb)
                    state[g] = (s_tiles, mpart)

                def stage_e(g):
                    """negmax + exp + row sums + 1/(sum*sqrt(hs)) for block g."""
                    tl = key_tiles(g)
                    s_tiles, mpart = state[g]
                    negm = ap.tile([128, 1], dt.float32, tag="negm", bufs=2,
                                   name="negm")
                    nc.vector.reduce_max(negm[:], mpart[:, 0:len(tl)], axis=Ax.X,
                                         negate=True)
                    esum = ap.tile([128, 4], dt.float32, tag="esum", bufs=2,
                                   name="esum")
                    e_tiles = []
                    for i, (col0, w) in enumerate(tl):
                        esb = ap.tile([128, w], dt.bfloat16, tag="esb", bufs=9,
                                      name="esb")
                        nc.scalar.activation(
                            esb[:], s_tiles[i][:], Act.Exp,
                            bias=negm[:], scale=1.0,
                            accum_out=esum[:, i:i + 1])
                        e_tiles.append(esb)
                    stot = ap.tile([128, 1], dt.float32, tag="stot", bufs=2,
                                   name="stot")
                    nc.vector.reduce_sum(stot[:], esum[:, 0:len(tl)], axis=Ax.X)
                    # 1/sqrt(head_size) is folded into PRJ host-side, so the
                    # normalizer is just 1/sum (one hop less before zn)
                    inv = ap.tile([128, 1], dt.float32, tag="inv", bufs=2,
                                  name="inv")
                    nc.vector.reciprocal(inv[:], stot[:])
                    state[g] = (e_tiles, inv)

                def stage_z_acc(g):
                    """E^T (plain matmuls vs identity), Z accumulation, zn."""
                    tl = key_tiles(g)
                    e_tiles, inv = state.pop(g)
                    ets = []
                    for (col0, w), esb in zip(tl, e_tiles):
                        for j in range(w // 128):
                            etp = psa.tile([128, 128], dt.float32, tag="etp",
                                           bufs=2, name="etp")
                            nc.tensor.matmul(
                                etp[:], esb[:, 128 * j:128 * j + 128],
                                ident[:], start=True, stop=True)
                            et = ap.tile([128, 128], dt.bfloat16, tag="ets",
                                         bufs=17, name="ets")
                            nc.vector.tensor_copy(et[:], etp[:])
                            ets.append((col0 // 128 + j, et))
                    zps = psa.tile([128, 512], dt.float32, tag="zps", bufs=2,
                                   name="zps")
                    for n, (kc, et) in enumerate(ets):
                        nc.tensor.matmul(
                            zps[:], et[:], V_sb[kc][:],
                            start=(n == 0), stop=(n == len(ets) - 1))
                    zn = ap.tile([128, 512], dt.bfloat16, tag="zn", bufs=2,
                                 name="zn")
                    nc.vector.tensor_scalar_mul(zn[:], zps[:], inv[:])
                    return zn

                def stage_z_tr(g, zn):
                    """Z^T via plain matmuls against the identity."""
                    zt = []
                    for j in range(4):
                        ztp = psa.tile([128, 128], dt.float32, tag="etp",
                                       name="ztp", bufs=2)
                        nc.tensor.matmul(ztp[:], zn[:, 128 * j:128 * j + 128],
                                         ident[:], start=True, stop=True)
                        zts = ap.tile([128, 128], dt.bfloat16, tag=f"zt{j}",
                                      bufs=2, name=f"zt{j}")
                        nc.vector.tensor_copy(zts[:], ztp[:])
                        zt.append(zts)
                    return zt

                def stage_o(g, zt, last=False):
                    """Output projection for block g + DMA out per 512-chunk.

                    Copies alternate ScalarE/VectorE and the DMAs alternate
                    sync/scalar queues (plus gpsimd for the final block) so
                    the last block's chain drains faster (shorter tail).
                    """
                    osb = ap.tile([128, D], dt.float32, tag="osb", bufs=2,
                                  name="osb")
                    for dtile in range(4):
                        ops = psa.tile([128, 512], dt.float32, tag="ops", bufs=2,
                                       name="ops")
                        for h in range(4):
                            nc.tensor.matmul(
                                ops[:], zt[h][:],
                                prj[h][:, 512 * dtile:512 * dtile + 512],
                                start=(h == 0), stop=(h == 3))
                        osl = osb[:, 512 * dtile:512 * dtile + 512]
                        if dtile % 2 == 0:
                            nc.scalar.copy(osl, ops[:])
                        else:
                            nc.vector.tensor_copy(osl, ops[:])
                        # halfwise DMA: 4KB rows into a contiguous block
                        if dtile == 1:
                            eng = nc.sync if g % 2 == 0 else nc.scalar
                            eng.dma_start(out=OUT_d[g][:, 0:1024],
                                          in_=osb[:, 0:1024])
                        elif dtile == 3:
                            eng = nc.scalar if g % 2 == 0 else nc.sync
                            if last:
                                eng = nc.gpsimd
                            eng.dma_start(out=OUT_d[g][:, 1024:2048],
                                          in_=osb[:, 1024:2048])

                # software pipeline: S(g+1)+O(g-1) matmuls hide the zn
                # normalize of block g; Z^T+O hide exp(g+1).
                stage_s(0)
                stage_e(0)
                zt_prev = None
                for g in range(RB):
                    zn = stage_z_acc(g)
                    if g + 1 < RB:
                        stage_s(g + 1)
                    if zt_prev is not None:
                        stage_o(g - 1, zt_prev)
                    zt_prev = stage_z_tr(g, zn)
                    if g + 1 < RB:
                        stage_e(g + 1)
                stage_o(RB - 1, zt_prev, last=True)

    nc.compile()
    return nc


def host_prep(x, WQ, bQ, WK, bK, WV, bV, proj):
    """Collapse weights, transpose layouts, build per-core input maps."""
    x = np.ascontiguousarray(x, dtype=np.float32)
    WQc = WQ.reshape(4, HD, D).sum(0)
    bQc = bQ.reshape(4, HD).sum(0)
    projc = proj.reshape(4, HD, D).sum(0)

    import ml_dtypes
    WQT = np.ascontiguousarray(WQc.T)               # [D, HD]
    WKT = np.ascontiguousarray(WK.T)
    WVT = np.ascontiguousarray(WV.T)
    PRJ = np.ascontiguousarray(
        projc / np.sqrt(np.float32(128.0))).astype(ml_dtypes.bfloat16)
    bVr = np.ascontiguousarray(
        np.broadcast_to(bV.reshape(1, HD), (128, HD)).astype(np.float32))
    one = np.ones((1, 128), dtype=np.float32)
    idt = np.eye(128).astype(ml_dtypes.bfloat16)
    # biases packed [128, 8]: cols 0-3 = bK chunks, cols 4-7 = bQc chunks
    bia = np.empty((128, 8), dtype=np.float32)
    for h in range(4):
        bia[:, h] = bK[128 * h:128 * h + 128]
        bia[:, 4 + h] = bQc[128 * h:128 * h + 128]

    def _tile_x(a):
        # [D, 512] (one key group) -> [4, 128, 2048], chunk c=4t+i at 512i
        return np.ascontiguousarray(
            a.reshape(4, 4, 128, 512).transpose(0, 2, 1, 3)
            .reshape(4, 128, 2048))

    in_maps = []
    for core in range(NCORES):
        b, q = divmod(core, 2)
        xT = np.ascontiguousarray(x[b].T)           # [D, T]
        rows = np.concatenate(
            [np.arange(256 * g + 128 * q, 256 * g + 128 * q + 128)
             for g in range(RB)])
        xTfull = np.concatenate(
            [_tile_x(xT[:, 512 * rt:512 * rt + 512]) for rt in range(4)],
            axis=0)                                  # [16, 128, 2048]
        xq = xT[:, rows]
        xTq = np.concatenate(
            [_tile_x(xq[:, 512 * rt:512 * rt + 512]) for rt in range(2)],
            axis=0)                                  # [8, 128, 2048]
        # causal mask of the last key tile depends only on block parity:
        # allowed key k (within tile) iff k <= p + off, off = 128*q + 256*(g%2)
        # (for even g the tail tile is 256 wide and uses msk[0][:, :256])
        msk = np.zeros((2, 128, 512), dtype=np.float32)
        for m in range(2):
            off = 128 * q + 256 * m
            key = np.arange(512)[None, :]
            row = (off + np.arange(128))[:, None]
            msk[m] = np.where(key <= row, 0.0, NEG)
        in_maps.append({
            "xT": xTfull, "xTq": xTq, "WKT": WKT, "WVT": WVT, "WQT": WQT,
            "PRJ": PRJ, "BIA": bia, "bVr": bVr, "ONE": one,
            "MSK": msk, "IDT": idt,
        })
    return in_maps


def assemble(results):
    """Gather per-core [1024, D] outputs into [B, T, D]."""
    y = np.empty((B, T, D), dtype=np.float32)
    for core in range(NCORES):
        b, q = divmod(core, 2)
        o = results[core]["out"]                     # [RB, 128, D]
        for g in range(RB):
            y[b, 256 * g + 128 * q:256 * g + 128 * q + 128] = o[g]
    return y


_NC_CACHE = None


def kernel(x, WQ, bQ, WK, bK, WV, bV, proj):
    global _NC_CACHE
    in_maps = host_prep(np.asarray(x), np.asarray(WQ), np.asarray(bQ),
                        np.asarray(WK), np.asarray(bK), np.asarray(WV),
                        np.asarray(bV), np.asarray(proj))
    if _NC_CACHE is None:
        _NC_CACHE = build_kernel()
    res = run_bass_kernel_spmd(_NC_CACHE, in_maps, list(range(NCORES)))
    return assemble(res.results)



# revision 16
# speedup vs baseline: 1.0137x; 1.0137x over previous
"""GroupedQueryAttention Trainium2 Bass kernel (8 NeuronCores, SPMD).

Reference quirk exploited: K/V are tiled R=4x along the group axis and
attention runs over the full concatenated 2048-dim. Mathematically this
collapses:
  scores = Q . tile(K)  ==  (sum of Q's four 512-chunks) . K      (512-dim)
  Z      = attn . tile(V)  -> tiled copies of  attn . V           (512-dim)
  out    = Z @ proj     ==  (attn . V) @ (sum of proj's 4 row-blocks)
So the whole module reduces to a single 512-dim attention:
  Qc = x @ WQc.T + bQc   (WQc = sum of WQ row-blocks)
  K  = x @ WK.T + bK ; V = x @ WV.T + bV
  S  = Qc K^T (causal), softmax, /sqrt(128)
  y  = (softmax(S)/sqrt(128) V) @ projc    (projc = sum of proj row-blocks)
This cuts FLOPs ~3x vs the literal graph.

Sharding: 8 cores = 4 batches x 2 (interleaved 128-row blocks). Core with
pairpos q of batch b owns rows {256g+128q .. 256g+128q+127, g=0..7}. The
causal key extent of block g is tiled at 256 granularity: (g+1)//2 full
512-tiles plus, for even g, a 256 tail. Both cores of a pair see the same
tile shapes -> a single SPMD program; the causal masks (two of them, by
block parity) are per-core input data.

All matmuls run as float32r (full-speed PE mode, ~tf32 accuracy).
Transposes are plain matmuls against a 128x128 identity moving operand
(out = lhsT.T @ I), ~3x faster than PE transpose-mode. Attention
row-blocks are software-pipelined: S(g+1) matmuls hide the zn normalize
of block g, Z^T+O(g) hide exp(g+1), and the output projection's DMA-out
overlaps the remaining blocks.
"""

import numpy as np

import concourse.bacc as bacc
import concourse.mybir as mybir
from concourse.tile import TileContext
from concourse.bass_utils import run_bass_kernel_spmd

B, T, D = 4, 2048, 2048
HD = 512                 # collapsed head dim
NCORES = 8
RB = 8                   # 128-row blocks per core
DCH = D // 128           # 16 contraction chunks
dt = mybir.dt
NEG = -1.0e30


def key_tiles(g):
    """Causal key tiles for row block g: [(col0, width), ...]."""
    t = [(512 * kt, 512) for kt in range((g + 1) // 2)]
    if g % 2 == 0:
        t.append((256 * g, 256))
    return t


def build_kernel():
    nc = bacc.Bacc(None, target_bir_lowering=False)

    # x streams pre-tiled host-side: tile 4*rt+t packs contraction chunks
    # c=4t+i at cols [512i, 512i+512) for key-group rt -> 8KB SBUF rows
    # (DMA queues move ~4x the bytes/packet vs 2KB rows).
    xT_d = nc.dram_tensor("xT", [16, 128, 2048], dt.float32r, kind="ExternalInput")
    xTq_d = nc.dram_tensor("xTq", [8, 128, 2048], dt.float32r, kind="ExternalInput")
    WKT_d = nc.dram_tensor("WKT", [D, HD], dt.float32r, kind="ExternalInput")
    WVT_d = nc.dram_tensor("WVT", [D, HD], dt.float32r, kind="ExternalInput")
    WQT_d = nc.dram_tensor("WQT", [D, HD], dt.float32r, kind="ExternalInput")
    PRJ_d = nc.dram_tensor("PRJ", [HD, D], dt.bfloat16, kind="ExternalInput")
    BIA_d = nc.dram_tensor("BIA", [128, 8], dt.float32, kind="ExternalInput")
    bVr_d = nc.dram_tensor("bVr", [128, HD], dt.float32, kind="ExternalInput")
    ONE_d = nc.dram_tensor("ONE", [1, 128], dt.float32r, kind="ExternalInput")
    MSK_d = nc.dram_tensor("MSK", [2, 128, 512], dt.float32, kind="ExternalInput")
    IDT_d = nc.dram_tensor("IDT", [128, 128], dt.bfloat16, kind="ExternalInput")
    OUT_d = nc.dram_tensor("out", [RB, 128, D], dt.float32, kind="ExternalOutput")

    Act = mybir.ActivationFunctionType
    Ax = mybir.AxisListType

    with TileContext(nc) as tc:
        with tc.tile_pool(name="persist", bufs=1) as pp:
            # ---- persistent tiles ------------------------------------------
            KT_sb = [pp.tile([128, T], dt.float32r, tag=f"KT{h}", name=f"KT{h}")
                     for h in range(4)]
            V_sb = [pp.tile([128, HD], dt.bfloat16, tag=f"V{k}", name=f"V{k}")
                    for k in range(16)]
            # Q^T split by 512-col group so S(0) doesn't wait on the rt=1 write
            QT_sb = [[pp.tile([128, 512], dt.float32r, tag=f"QT{h}{rt}",
                              name=f"QT{h}{rt}") for rt in range(2)]
                     for h in range(4)]
            ident = pp.tile([128, 128], dt.bfloat16, tag="ident")
            ones = pp.tile([1, 128], dt.float32r, tag="ones")
            bvr = pp.tile([128, HD], dt.float32, tag="bvr")
            bia = pp.tile([128, 8], dt.float32, tag="bia")
            msk_sb = [pp.tile([128, 512], dt.float32, tag=f"msk{m}",
                              name=f"msk{m}") for m in range(2)]
            prj = [pp.tile([128, D], dt.bfloat16, tag=f"prj{h}", name=f"prj{h}")
                   for h in range(4)]

            def bias_ap(nm, h):
                col = h if nm == "bK" else 4 + h
                return bia[:, col:col + 1]

            # ---- stage P: projections --------------------------------------
            # x^T streamed by 128-D chunk; weights streamed with the first key
            # group then resident. Per 512-key group: 4 PSUM banks accumulate
            # K^T hd-tiles ([hd, keys], bias per partition) and 4 banks
            # accumulate V key-blocks directly in [keys, hd] layout (xt chunk
            # as lhsT); V bias enters as a rank-1 ones x bV matmul. PSUM
            # drains are split between ScalarE and VectorE.
            with tc.tile_pool(name="wpool", bufs=1) as wp, \
                 tc.tile_pool(name="xstream", bufs=3) as xp, \
                 tc.tile_pool(name="psP", bufs=1, space="PSUM") as psp:
                wk = [None] * DCH
                wv = [None] * DCH
                for rt in range(4):                      # key 512-col groups
                    kps = [psp.tile([128, 512], dt.float32, tag=f"kps{h}",
                                    name=f"kps{h}") for h in range(4)]
                    vps = [psp.tile([128, 512], dt.float32, tag=f"vps{j}",
                                    name=f"vps{j}") for j in range(4)]
                    for t in range(4):
                        xt = xp.tile([128, 2048], dt.float32r, tag="xs", name="xt")
                        if rt == 0 and t == 0:
                            nc.gpsimd.dma_start(out=xt[:, 0:512],
                                                in_=xT_d[0][:, 0:512])
                            nc.gpsimd.dma_start(out=xt[:, 512:2048],
                                                in_=xT_d[0][:, 512:2048])
                        else:
                            nc.gpsimd.dma_start(out=xt[:], in_=xT_d[4 * rt + t])
                        for i in range(4):
                            c = 4 * t + i
                            if rt == 0:   # stream weights in with first pass
                                tk = wp.tile([128, HD], dt.float32r, tag=f"wk{c}",
                                             name=f"wk{c}")
                                nc.sync.dma_start(
                                    out=tk[:], in_=WKT_d[128 * c:128 * c + 128, :])
                                wk[c] = tk
                                tv = wp.tile([128, HD], dt.float32r, tag=f"wv{c}",
                                             name=f"wv{c}")
                                nc.sync.dma_start(
                                    out=tv[:], in_=WVT_d[128 * c:128 * c + 128, :])
                                wv[c] = tv
                                if c == 0:
                                    # tiny constants on the idle scalar queue
                                    nc.scalar.dma_start(out=bia[:], in_=BIA_d[:])
                                    nc.scalar.dma_start(out=ones[:], in_=ONE_d[:])
                                    nc.scalar.dma_start(out=bvr[:], in_=bVr_d[:])
                            st = (c == 0)
                            xsl = xt[:, 512 * i:512 * i + 512]
                            for h in range(4):
                                nc.tensor.matmul(kps[h][:],
                                                 wk[c][:, 128 * h:128 * h + 128],
                                                 xsl, start=st,
                                                 stop=(c == DCH - 1))
                            for j in range(4):
                                nc.tensor.matmul(
                                    vps[j][:],
                                    xt[:, 512 * i + 128 * j:
                                       512 * i + 128 * j + 128],
                                    wv[c][:], start=st,
                                    stop=(c == DCH - 1))
                    if rt == 0:
                        # prefetches ride the DMA slack (first use much later)
                        nc.sync.dma_start(out=ident[:], in_=IDT_d[:])
                        for m in range(2):
                            nc.sync.dma_start(out=msk_sb[m][:], in_=MSK_d[m])
                        for h in range(4):
                            nc.sync.dma_start(out=prj[h][:],
                                              in_=PRJ_d[128 * h:128 * h + 128, :])
                    for h in range(4):
                        if h % 2 == 0:
                            nc.scalar.activation(
                                KT_sb[h][:, 512 * rt:512 * rt + 512], kps[h][:],
                                Act.Identity, bias=bias_ap("bK", h), scale=1.0)
                        else:
                            nc.vector.tensor_scalar_add(
                                KT_sb[h][:, 512 * rt:512 * rt + 512], kps[h][:],
                                bias_ap("bK", h))
                    for j in range(4):   # V bias folded into the drain
                        nc.vector.tensor_add(V_sb[4 * rt + j][:], vps[j][:],
                                             bvr[:])
                # Qc^T for this core's 1024 rows (reuses the wk slots).
                wq = []
                for c in range(DCH):
                    tq = wp.tile([128, HD], dt.float32r, tag=f"wk{c}", name=f"wq{c}")
                    nc.sync.dma_start(out=tq[:], in_=WQT_d[128 * c:128 * c + 128, :])
                    wq.append(tq)
                for rt in range(2):
                    qps = [psp.tile([128, 512], dt.float32, tag=f"kps{h}",
                                    name=f"qps{h}") for h in range(4)]
                    for t in range(4):
                        xt = xp.tile([128, 2048], dt.float32r, tag="xs", name="xt")
                        nc.gpsimd.dma_start(out=xt[:], in_=xTq_d[4 * rt + t])
                        for i in range(4):
                            c = 4 * t + i
                            for h in range(4):
                                nc.tensor.matmul(qps[h][:],
                                                 wq[c][:, 128 * h:128 * h + 128],
                                                 xt[:, 512 * i:512 * i + 512],
                                                 start=(c == 0),
                                                 stop=(c == DCH - 1))
                    for h in range(4):
                        if h % 2 == 0:
                            nc.scalar.activation(
                                QT_sb[h][rt][:], qps[h][:],
                                Act.Identity, bias=bias_ap("bQ", h), scale=1.0)
                        else:
                            nc.vector.tensor_scalar_add(
                                QT_sb[h][rt][:], qps[h][:], bias_ap("bQ", h))

            # ---- stages A+O: attention fused with output projection --------
            with tc.tile_pool(name="attn", bufs=1) as ap, \
                 tc.tile_pool(name="psA", bufs=1, space="PSUM") as psa:
                state = {}

                def qt_ap(h, g):
                    return QT_sb[h][g // 4][:, 128 * (g % 4):128 * (g % 4) + 128]

                def stage_s(g):
                    """S matmuls + mask + per-tile max for row block g."""
                    tl = key_tiles(g)
                    mpart = ap.tile([128, 4], dt.float32, tag="mpart", bufs=2,
                                    name="mpart")
                    s_tiles = []
                    for i, (col0, w) in enumerate(tl):
                        sps = psa.tile([128, w], dt.float32, tag="sps", bufs=2,
                                       name="sps")
                        for h in range(4):
                            nc.tensor.matmul(
                                sps[:], qt_ap(h, g),
                                KT_sb[h][:, col0:col0 + w],
                                start=(h == 0), stop=(h == 3))
                        # exp reads these as fp32r (same bits) so the E^T
                        # transpose can be a plain full-speed matmul
                        ssb = ap.tile([128, w], dt.float32, tag="ssb", bufs=9,
                                      name="ssb")
                        if i == len(tl) - 1:
                            nc.vector.tensor_add(ssb[:], sps[:],
                                                 msk_sb[g % 2][:, 0:w])
                        else:
                            nc.scalar.copy(ssb[:], sps[:])
                        nc.vector.reduce_max(mpart[:, i:i + 1], ssb[:],
                                             axis=Ax.X)
                        s_tiles.append(ssb)
                    state[g] = (s_tiles, mpart)

                def stage_e(g):
                    """negmax + exp + row sums + 1/(sum*sqrt(hs)) for block g."""
                    tl = key_tiles(g)
                    s_tiles, mpart = state[g]
                    negm = ap.tile([128, 1], dt.float32, tag="negm", bufs=2,
                                   name="negm")
                    nc.vector.reduce_max(negm[:], mpart[:, 0:len(tl)], axis=Ax.X,
                                         negate=True)
                    esum = ap.tile([128, 4], dt.float32, tag="esum", bufs=2,
                                   name="esum")
                    e_tiles = []
                    for i, (col0, w) in enumerate(tl):
                        esb = ap.tile([128, w], dt.bfloat16, tag="esb", bufs=9,
                                      name="esb")
                        nc.scalar.activation(
                            esb[:], s_tiles[i][:], Act.Exp,
                            bias=negm[:], scale=1.0,
                            accum_out=esum[:, i:i + 1])
                        e_tiles.append(esb)
                    stot = ap.tile([128, 1], dt.float32, tag="stot", bufs=2,
                                   name="stot")
                    nc.vector.reduce_sum(stot[:], esum[:, 0:len(tl)], axis=Ax.X)
                    # 1/sqrt(head_size) is folded into PRJ host-side, so the
                    # normalizer is just 1/sum (one hop less before zn)
                    inv = ap.tile([128, 1], dt.float32, tag="inv", bufs=2,
                                  name="inv")
                    nc.vector.reciprocal(inv[:], stot[:])
                    state[g] = (e_tiles, inv)

                def stage_z_acc(g):
                    """E^T (plain matmuls vs identity), Z accumulation, zn."""
                    tl = key_tiles(g)
                    e_tiles, inv = state.pop(g)
                    ets = []
                    for (col0, w), esb in zip(tl, e_tiles):
                        for j in range(w // 128):
                            etp = psa.tile([128, 128], dt.float32, tag="etp",
                                           bufs=2, name="etp")
                            nc.tensor.matmul(
                                etp[:], esb[:, 128 * j:128 * j + 128],
                                ident[:], start=True, stop=True)
                            et = ap.tile([128, 128], dt.bfloat16, tag="ets",
                                         bufs=17, name="ets")
                            nc.vector.tensor_copy(et[:], etp[:])
                            ets.append((col0 // 128 + j, et))
                    zps = psa.tile([128, 512], dt.float32, tag="zps", bufs=2,
                                   name="zps")
                    for n, (kc, et) in enumerate(ets):
                        nc.tensor.matmul(
                            zps[:], et[:], V_sb[kc][:],
                            start=(n == 0), stop=(n == len(ets) - 1))
                    zn = ap.tile([128, 512], dt.bfloat16, tag="zn", bufs=2,
                                 name="zn")
                    nc.vector.tensor_scalar_mul(zn[:], zps[:], inv[:])
                    return zn

                def stage_z_tr(g, zn):
                    """Z^T via plain matmuls against the identity."""
                    zt = []
                    for j in range(4):
                        ztp = psa.tile([128, 128], dt.float32, tag="etp",
                                       name="ztp", bufs=2)
                        nc.tensor.matmul(ztp[:], zn[:, 128 * j:128 * j + 128],
                                         ident[:], start=True, stop=True)
                        zts = ap.tile([128, 128], dt.bfloat16, tag=f"zt{j}",
                                      bufs=2, name=f"zt{j}")
                        nc.vector.tensor_copy(zts[:], ztp[:])
                        zt.append(zts)
                    return zt

                def stage_o(g, zt, last=False):
                    """Output projection for block g + DMA out per 512-chunk.

                    Copies alternate ScalarE/VectorE and the DMAs alternate
                    sync/scalar queues (plus gpsimd for the final block) so
                    the last block's chain drains faster (shorter tail).
                    """
                    osb = ap.tile([128, D], dt.float32, tag="osb", bufs=2,
                                  name="osb")
                    for dtile in range(4):
                        ops = psa.tile([128, 512], dt.float32, tag="ops", bufs=2,
                                       name="ops")
                        for h in range(4):
                            nc.tensor.matmul(
                                ops[:], zt[h][:],
                                prj[h][:, 512 * dtile:512 * dtile + 512],
                                start=(h == 0), stop=(h == 3))
                        osl = osb[:, 512 * dtile:512 * dtile + 512]
                        if dtile % 2 == 0:
                            nc.scalar.copy(osl, ops[:])
                        else:
                            nc.vector.tensor_copy(osl, ops[:])
                        # halfwise DMA: 4KB rows into a contiguous block
                        if dtile == 1:
                            eng = nc.sync if g % 2 == 0 else nc.scalar
                            eng.dma_start(out=OUT_d[g][:, 0:1024],
                                          in_=osb[:, 0:1024])
                        elif dtile == 3:
                            eng = nc.scalar if g % 2 == 0 else nc.sync
                            if last:
                                eng = nc.gpsimd
                            eng.dma_start(out=OUT_d[g][:, 1024:2048],
                                          in_=osb[:, 1024:2048])

                # software pipeline: S(g+1)+O(g-1) matmuls hide the zn
                # normalize of block g; Z^T+O hide exp(g+1).
                stage_s(0)
                stage_e(0)
                zt_prev = None
                for g in range(RB):
                    zn = stage_z_acc(g)
                    if g + 1 < RB:
                        stage_s(g + 1)
                    if zt_prev is not None:
                        stage_o(g - 1, zt_prev)
                    zt_prev = stage_z_tr(g, zn)
                    if g + 1 < RB:
                        stage_e(g + 1)
                stage_o(RB - 1, zt_prev, last=True)

    nc.compile()
    return nc


def host_prep(x, WQ, bQ, WK, bK, WV, bV, proj):
    """Collapse weights, transpose layouts, build per-core input maps."""
    x = np.ascontiguousarray(x, dtype=np.float32)
    WQc = WQ.reshape(4, HD, D).sum(0)
    bQc = bQ.reshape(4, HD).sum(0)
    projc = proj.reshape(4, HD, D).sum(0)

    import ml_dtypes
    WQT = np.ascontiguousarray(WQc.T)               # [D, HD]
    WKT = np.ascontiguousarray(WK.T)
    WVT = np.ascontiguousarray(WV.T)
    PRJ = np.ascontiguousarray(
        projc / np.sqrt(np.float32(128.0))).astype(ml_dtypes.bfloat16)
    bVr = np.ascontiguousarray(
        np.broadcast_to(bV.reshape(1, HD), (128, HD)).astype(np.float32))
    one = np.ones((1, 128), dtype=np.float32)
    idt = np.eye(128).astype(ml_dtypes.bfloat16)
    # biases packed [128, 8]: cols 0-3 = bK chunks, cols 4-7 = bQc chunks
    bia = np.empty((128, 8), dtype=np.float32)
    for h in range(4):
        bia[:, h] = bK[128 * h:128 * h + 128]
        bia[:, 4 + h] = bQc[128 * h:128 * h + 128]

    def _tile_x(a):
        # [D, 512] (one key group) -> [4, 128, 2048], chunk c=4t+i at 512i
        return np.ascontiguousarray(
            a.reshape(4, 4, 128, 512).transpose(0, 2, 1, 3)
            .reshape(4, 128, 2048))

    in_maps = []
    for core in range(NCORES):
        b, q = divmod(core, 2)
        xT = np.ascontiguousarray(x[b].T)           # [D, T]
        rows = np.concatenate(
            [np.arange(256 * g + 128 * q, 256 * g + 128 * q + 128)
             for g in range(RB)])
        xTfull = np.concatenate(
            [_tile_x(xT[:, 512 * rt:512 * rt + 512]) for rt in range(4)],
            axis=0)                                  # [16, 128, 2048]
        xq = xT[:, rows]
        xTq = np.concatenate(
            [_tile_x(xq[:, 512 * rt:512 * rt + 512]) for rt in range(2)],
            axis=0)                                  # [8, 128, 2048]
        # causal mask of the last key tile depends only on block parity:
        # allowed key k (within tile) iff k <= p + off, off = 128*q + 256*(g%2)
        # (for even g the tail tile is 256 wide and uses msk[0][:, :256])
        msk = np.zeros((2, 128, 512), dtype=np.float32)
        for m in range(2):
            off = 128 * q + 256 * m
            key = np.arange(512)[None, :]
            row = (off + np.arange(128))[:, None]
            msk[m] = np.where(key <= row, 0.0, NEG)
        in_maps.append({
            "xT": xTfull, "xTq": xTq, "WKT": WKT, "WVT": WVT, "WQT": WQT,
            "PRJ": PRJ, "BIA": bia, "bVr": bVr, "ONE": one,
            "MSK": msk, "IDT": idt,
        })
    return in_maps


def assemble(results):
    """Gather per-core [1024, D] outputs into [B, T, D]."""
    y = np.empty((B, T, D), dtype=np.float32)
    for core in range(NCORES):
        b, q = divmod(core, 2)
        o = results[core]["out"]                     # [RB, 128, D]
        for g in range(RB):
            y[b, 256 * g + 128 * q:256 * g + 128 * q + 128] = o[g]
    return y


_NC_CACHE = None


def kernel(x, WQ, bQ, WK, bK, WV, bV, proj):
    global _NC_CACHE
    in_maps = host_prep(np.asarray(x), np.asarray(WQ), np.asarray(bQ),
                        np.asarray(WK), np.asarray(bK), np.asarray(WV),
                        np.asarray(bV), np.asarray(proj))
    if _NC_CACHE is None:
        _NC_CACHE = build_kernel()
    res = run_bass_kernel_spmd(_NC_CACHE, in_maps, list(range(NCORES)))
    return assemble(res.results)

